# revision 22
# baseline (speedup 1.0000x reference)
"""Causal self-attention (GQA, RoPE) Trainium2 Bass kernel.

Full inputs in, full output out. Tensor-parallel over heads across 8
NeuronCores: core i computes q-heads 4i..4i+3 (kv head i) and a partial
output projection over its 256 attn-out features; the host sums the 8
partial outputs (the "all-reduce after output_proj" step).

v2 design notes (vs the transpose-heavy v1):
- x is passed pre-transposed from the host (xT [E,S]), so the qkv
  projection needs no on-device x transposes.
- Scores are computed directly in transposed form S^T[k,q] =
  (kT)^T @ qT, so the attention matrix never needs a PE transpose and
  exp() output feeds the AV matmul as-is.
- The causal mask is a multiplicative 0/1 mask applied after exp on the
  Pool engine (otherwise idle).
- Softmax denominators come for free from the AV matmul: the stationary
  V operand carries an extra all-ones column ([v|1] for even heads,
  [1|v] at partition offset 63 for odd heads), so row sums accumulate
  in PSUM alongside the AV product. Normalization then happens on the
  [64,512] AV output (per-q reciprocal broadcast via a rank-1 matmul),
  not on the full [q,k] attention matrix.
"""

import numpy as np

import concourse.bacc as bacc
import concourse.mybir as mybir
import concourse.tile as tile
from concourse.bass_utils import run_bass_kernel_spmd

S = 2048          # sequence length
E = 2048          # embedding dim
H = 32            # query heads
KV = 8            # kv heads
HD = 64           # head dim
NCORES = 8
HC = H // NCORES  # query heads per core = 4
DQ = HC * HD      # per-core q proj width = 256
DKV = HD          # per-core kv proj width = 64
DQK = DQ + DKV    # roped span = 320
DW = DQ + 2 * DKV  # fused qkv proj width = 384
ST = S // 128     # 16 s-tiles of 128 rows
VW = DKV + 1      # v storage width per s-tile: [v | ones] = 65

F32 = mybir.dt.float32
F32R = mybir.dt.float32r

EXPF = mybir.ActivationFunctionType.Exp


def r(ap):
    """Bitcast an AP to float32r so the PE runs fast-mode fp32 matmuls."""
    return ap.bitcast(F32R)


def build_nc(seq_tiles=ST, reps=1, phases=(1, 2, 3)):
    """Build + compile the per-core Bass program (identical on all cores)."""
    st_n = seq_tiles
    s_n = st_n * 128
    qb_n = s_n // 512

    nc = bacc.Bacc("TRN2", target_bir_lowering=False, debug=False)
    xt_d = nc.dram_tensor("xt", [E, s_n], F32R, kind="ExternalInput")
    wt_d = nc.dram_tensor("wt", [E, DW], F32R, kind="ExternalInput")
    wot_d = nc.dram_tensor("wot", [DQ, E], F32R, kind="ExternalInput")
    cos_d = nc.dram_tensor("cosh", [s_n, DQK // 2], F32, kind="ExternalInput")
    sin_d = nc.dram_tensor("sinh", [s_n, DQK // 2], F32, kind="ExternalInput")
    mask_d = nc.dram_tensor("mask01", [128, 128], F32R, kind="ExternalInput")
    id_d = nc.dram_tensor("ident", [128, 128], F32R, kind="ExternalInput")
    out_d = nc.dram_tensor("out", [s_n, E], F32, kind="ExternalOutput")

    with tile.TileContext(nc) as tc, nc.allow_low_precision(
        reason="bf16 transpose staging; all matmul accumulation stays fp32"
    ):
        for _rep in range(reps):
            # ---------- persistent constants / cross-phase tensors ----------
            with (
                tc.tile_pool(name="const", bufs=1) as constp,
                tc.tile_pool(name="qkv_store", bufs=1) as storep,
            ):
                ident = constp.tile([128, 128], F32R)
                nc.sync.dma_start(out=ident[:], in_=id_d.ap()[:, :])

                # phase-2/3 constants (loaded behind the phase-1-critical DMAs)
                woT_sb = constp.tile([128, 2, E], F32R)
                # multiplicative causal mask for the [128,128] diagonal
                # sub-block of an S^T chunk: mask_sb[i, j] = (i <= j)
                mask_sb = constp.tile([128, 128], F32R)
                ones_sb = constp.tile([128, 64], F32R)
                nc.vector.memset(ones_sb[:], 1.0)

                # qT: all heads on partitions 0:64; head h of s-tile t in
                # cols t*512 + h*128.
                qT_sb = storep.tile([64, st_n * 512], F32R)
                # kT: kv head on partitions 0:64.
                kT_sb = storep.tile([64, s_n], F32R)
                # v: [s, d] per s-tile stored as [v(64) | ones(1) | v(64)]:
                # even heads take cols 0:65 = [v|1], odd heads cols 64:129 =
                # [1|v], so the AV matmul emits rowsums next to the d-block
                # each parity needs.
                v_sb = storep.tile([128, st_n, VW], F32R)
                nc.vector.memset(v_sb[:, :, DKV:DKV + 1], 1.0)
                # attn-out transposed: head-pair hp in col block hp*s_n.
                aoT_sb = storep.tile([128, 2 * s_n], F32R)

                # ================= phase 1: qkv proj + rope =================
                with (
                    tc.tile_pool(name="p1_sbuf", bufs=2) as p1,
                    tc.tile_pool(name="p1_w", bufs=2) as p1w,
                    tc.tile_pool(name="p1_ps_qkv", bufs=2, space="PSUM") as ps_qkv_p,
                    tc.tile_pool(name="p1_ps_tr", bufs=2, space="PSUM") as ps_tr_p,
                ):
                    wT_sb = p1w.tile([128, E // 128, DW], F32R)
                    for j in range(E // 128):
                        nc.sync.dma_start(
                            out=wT_sb[:, j, :],
                            in_=wt_d.ap()[j * 128:(j + 1) * 128, :],
                        )

                    xt_v = xt_d.ap().rearrange("(c p) s -> p c s", p=128)
                    for t in range(st_n if 1 in phases else 0):
                        xT_sb = p1.tile([128, E // 128, 128], F32R, tag="x")
                        nc.sync.dma_start(
                            out=xT_sb[:], in_=xt_v[:, :, t * 128:(t + 1) * 128]
                        )
                        cs_sb = p1.tile([128, 2, DQK // 2], F32, tag="cs")
                        nc.sync.dma_start(
                            out=cs_sb[:, 0, :], in_=cos_d.ap()[t * 128:(t + 1) * 128, :]
                        )
                        nc.sync.dma_start(
                            out=cs_sb[:, 1, :], in_=sin_d.ap()[t * 128:(t + 1) * 128, :]
                        )
                        ps_qkv = ps_qkv_p.tile([128, DW], F32, tag="qkv")
                        for j in range(E // 128):
                            nc.tensor.matmul(
                                ps_qkv[:],
                                r(xT_sb[:, j, :]),
                                r(wT_sb[:, j, :]),
                                start=(j == 0),
                                stop=(j == E // 128 - 1),
                            )

                        # ---- rope on q+k jointly (320 cols); copy v ----
                        pairs = DQK // 2  # 160
                        qk_sb = p1.tile([128, DQK], F32R, tag="qkro")
                        se = ps_qkv[:, 0:DQK].rearrange("p (n two) -> p two n", two=2)
                        de = qk_sb[:].rearrange("p (n two) -> p two n", two=2)
                        c_ap = cs_sb[:, 0, :]
                        s_ap = cs_sb[:, 1, :]
                        t1 = p1.tile([128, pairs], F32, tag="t1")
                        t2 = p1.tile([128, pairs], F32, tag="t2")
                        nc.vector.tensor_mul(t1[:], se[:, 0, :], c_ap)
                        nc.vector.tensor_mul(t2[:], se[:, 1, :], s_ap)
                        nc.vector.tensor_sub(de[:, 0, :], t1[:], t2[:])
                        t3 = p1.tile([128, pairs], F32, tag="t3")
                        t4 = p1.tile([128, pairs], F32, tag="t4")
                        nc.vector.tensor_mul(t3[:], se[:, 1, :], c_ap)
                        nc.vector.tensor_mul(t4[:], se[:, 0, :], s_ap)
                        nc.vector.tensor_add(de[:, 1, :], t3[:], t4[:])

                        nc.vector.tensor_copy(v_sb[:, t, 0:DKV], ps_qkv[:, DQK:DW])

                        # ---- transpose roped q/k into qT/kT (partitions 0:64) ----
                        ps_trq = ps_tr_p.tile([64, 512], BF16, tag="trq")
                        for hh in range(4):
                            nc.tensor.matmul(
                                r(ps_trq[:, hh * 128:(hh + 1) * 128]),
                                r(qk_sb[:, hh * 64:(hh + 1) * 64]),
                                r(ident[:]),
                                is_transpose=True,
                                start=(hh == 0),
                                stop=(hh == 3),
                            )
                        nc.vector.tensor_copy(qT_sb[:, t * 512:(t + 1) * 512], ps_trq[:])
                        ps_trk_t = ps_tr_p.tile([64, 512], BF16, tag="trq")
                    ps_trk = ps_trk_t[:, 0:128]
                        nc.tensor.matmul(
                            r(ps_trk), r(qk_sb[:, 256:DQK]), r(ident[:]),
                            is_transpose=True, start=True, stop=True,
                        )
                        nc.vector.tensor_copy(kT_sb[:, t * 128:(t + 1) * 128], ps_trk)

                # ================= phase 2: attention =================
                qT_v = qT_sb[:].rearrange("p (t h c) -> p t h c", h=HC, c=128)
                with (
                    tc.tile_pool(name="p2_at", bufs=2) as p2t,
                    tc.tile_pool(name="p2_small", bufs=4) as p2s,
                    tc.tile_pool(name="p3_o", bufs=2) as p3o,
                    tc.tile_pool(name="p2_ps_s", bufs=2, space="PSUM") as ps_s_p,
                    tc.tile_pool(name="p2_ps_av", bufs=2, space="PSUM") as ps_av_p,
                    tc.tile_pool(name="p2_ps_rb", bufs=2, space="PSUM") as ps_rb_p,
                    tc.tile_pool(name="p3_ps", bufs=2, space="PSUM") as ps_o_p,
                ):
                    # deferred constant loads: queued behind phase-1 DMAs,
                    # consumed from qb=0's mask / phase-3 onward.
                    nc.sync.dma_start(out=mask_sb[:], in_=mask_d.ap()[:, :])
                    nc.sync.dma_start(
                        out=woT_sb[:],
                        in_=wot_d.ap().rearrange("(c p) e -> p c e", p=128),
                    )
                    for qb in range(qb_n if 2 in phases else 0):
                        nch = 4 * qb + 4  # causal 128-chunks for this q block
                        for h in range(HC):
                            odd = h & 1
                            p0 = 64 * odd
                            hp2 = h >> 1
                            # A~^T for this (qb, h): chunk kc in cols kc*512.
                            # Diagonal chunks only live in cols >= lo; the
                            # fully-masked left part is never written or read.
                            aT = p2t.tile([128, st_n * 512], F32R, tag="aT")
                            for kc in range(nch):
                                dk = kc - 4 * qb
                                lo = max(0, dk) * 128
                                ps_sT = ps_s_p.tile([128, 512], F32, tag="sT")
                                nc.tensor.matmul(
                                    ps_sT[:, lo:512],
                                    r(kT_sb[:, kc * 128:(kc + 1) * 128]),
                                    r(qT_v[:, 4 * qb + max(0, dk):4 * qb + 4, h, :]),
                                    start=True,
                                    stop=True,
                                )
                                nc.scalar.activation(
                                    aT[:, kc * 512 + lo:(kc + 1) * 512],
                                    ps_sT[:, lo:512],
                                    EXPF,
                                    scale=0.125,
                                )
                                if dk >= 0:
                                    # triangular 0/1 mask on the [128,128]
                                    # diagonal sub-block (cols beyond it are
                                    # fully visible, cols below lo unused)
                                    nc.gpsimd.tensor_mul(
                                        aT[:, kc * 512 + lo:kc * 512 + lo + 128],
                                        aT[:, kc * 512 + lo:kc * 512 + lo + 128],
                                        mask_sb[:],
                                    )
                            # ---- AV + rowsums: out^T[d, q] over k chunks ----
                            # stationary [v|1]: d @ parts 0:64, softmax
                            # denominators accumulate @ partition 64 for free.
                            ps_av = ps_av_p.tile([128, 512], F32, tag="av")
                            for kc in range(nch):
                                lo = max(0, kc - 4 * qb) * 128
                                nc.tensor.matmul(
                                    ps_av[0:DKV + 1, lo:512],
                                    r(v_sb[:, kc, :]),
                                    r(aT[:, kc * 512 + lo:(kc + 1) * 512]),
                                    start=(kc == 0),
                                    stop=(kc == nch - 1),
                                )
                            # ---- normalize: rinv broadcast via rank-1 matmul ----
                            rinv = p2s.tile([128, 512], F32, tag="rinv")
                            nc.vector.reciprocal(rinv[64:65, :], ps_av[64:65, :])
                            ps_rb = ps_rb_p.tile([128, 512], F32, tag="rb")
                            nc.tensor.matmul(
                                ps_rb[0:64, :],
                                r(ones_sb[64:65, :]),
                                r(rinv[64:65, :]),
                                start=True,
                                stop=True,
                            )
                            dst = aoT_sb[0:64,
                                         hp2 * s_n + qb * 512:hp2 * s_n + (qb + 1) * 512]
                            if not odd:
                                nc.vector.tensor_mul(
                                    dst, ps_av[0:64, :], ps_rb[0:64, :]
                                )
                            else:
                                # odd heads live at partitions 64:128 of aoT;
                                # DVE can't cross lanes, so normalize at 0:64
                                # then shift up via an identity matmul.
                                stg = p2s.tile([64, 512], F32R, tag="stg")
                                nc.vector.tensor_mul(
                                    stg[:], ps_av[0:64, :], ps_rb[0:64, :]
                                )
                                nc.tensor.matmul(
                                    ps_rb[64:128, :],
                                    r(ident[0:64, 0:64]),
                                    r(stg[:]),
                                    start=True,
                                    stop=True,
                                )
                                nc.vector.tensor_copy(
                                    aoT_sb[64:128,
                                           hp2 * s_n + qb * 512:hp2 * s_n + (qb + 1) * 512],
                                    ps_rb[64:128, :],
                                )

                        # ---- phase 3 for this q block: output projection ----
                        for st in range(4 * qb, (4 * qb + 4) if 3 in phases else 4 * qb):
                            o_sb = p3o.tile([128, E], F32, tag="o")
                            for eb in range(E // 512):
                                ps_o = ps_o_p.tile([128, 512], F32, tag="po")
                                for c in range(2):
                                    nc.tensor.matmul(
                                        ps_o[:],
                                        r(aoT_sb[:, c * s_n + st * 128:c * s_n + (st + 1) * 128]),
                                        r(woT_sb[:, c, eb * 512:(eb + 1) * 512]),
                                        start=(c == 0),
                                        stop=(c == 1),
                                    )
                                eng = nc.vector if eb % 2 == 0 else nc.gpsimd
                                eng.tensor_copy(
                                    o_sb[:, eb * 512:(eb + 1) * 512], ps_o[:]
                                )
                            nc.sync.dma_start(
                                out=out_d.ap()[st * 128:(st + 1) * 128, :], in_=o_sb[:]
                            )

    nc.compile()
    return nc


def make_tables(s_n=S):
    """Host-side RoPE tables and multiplicative causal mask (transposed)."""
    theta = (1.0 / (10000.0 ** (np.arange(0, HD, 2, dtype=np.float32) / HD))).astype(
        np.float32
    )
    freqs = np.arange(s_n, dtype=np.float32)[:, None] * theta[None, :]  # [s, 32]
    cos = np.cos(freqs).astype(np.float32)
    sin = np.sin(freqs).astype(np.float32)
    cosh = np.tile(cos, (1, DQK // HD))  # [s, 160]
    sinh = np.tile(sin, (1, DQK // HD))
    # triangular 0/1 mask for a [128,128] diagonal sub-block: i <= j
    i = np.arange(128)[:, None]
    j = np.arange(128)[None, :]
    mask01 = (i <= j).astype(np.float32)
    return cosh, sinh, mask01


def make_core_inputs(x2, wq, wk, wv, wo, core):
    """Per-core input dict (host-side sharding prep)."""
    cosh, sinh, mask01 = _TABLES
    i = core
    wq_i = wq[i * DQ:(i + 1) * DQ]
    wk_i = wk[i * DKV:(i + 1) * DKV]
    wv_i = wv[i * DKV:(i + 1) * DKV]
    wt = np.ascontiguousarray(np.concatenate([wq_i, wk_i, wv_i], axis=0).T)
    wot = np.ascontiguousarray(wo[:, i * DQ:(i + 1) * DQ].T)
    return {
        "xt": _get_xt(x2),
        "wt": wt.astype(NP_BF16),
        "wot": wot.astype(NP_BF16),
        "cosh": cosh,
        "sinh": sinh,
        "mask01": mask01.astype(NP_BF16),
        "ident": np.eye(128, dtype=NP_BF16),
    }


_TABLES = make_tables()
_NC_CACHE = {}
_XT_CACHE = {}


def _get_xt(x2):
    key = id(x2)
    if _XT_CACHE.get("key") != key:
        _XT_CACHE["key"] = key
        _XT_CACHE["xt"] = np.ascontiguousarray(x2.T).astype(NP_BF16)
    return _XT_CACHE["xt"]


def _get_nc(reps=1):
    key = ("nc", reps)
    if key not in _NC_CACHE:
        _NC_CACHE[key] = build_nc(reps=reps)
    return _NC_CACHE[key]


def kernel(x, wq, wk, wv, wo):
    x = np.asarray(x, dtype=np.float32)
    b, s_n, e = x.shape
    x2 = np.ascontiguousarray(x.reshape(s_n, e))
    in_maps = [
        make_core_inputs(x2, np.asarray(wq, np.float32), np.asarray(wk, np.float32),
                         np.asarray(wv, np.float32), np.asarray(wo, np.float32), i)
        for i in range(NCORES)
    ]
    res = run_bass_kernel_spmd(_get_nc(), in_maps, core_ids=list(range(NCORES)))
    out = np.zeros((s_n, e), dtype=np.float32)
    for rr in res.results:
        out += rr["out"]
    return out.reshape(b, s_n, e).astype(np.float32)
